# revision 45
# baseline (speedup 1.0000x reference)
"""Trainium2 Bass kernel for the GNN message-passing model.

Data-parallel over molecules: 32 molecules -> 8 NeuronCores x 4 molecules.
Each core runs the full per-molecule pipeline; no collectives needed.
Matmuls run in fp16 (fp32 PSUM accumulate); distances/top-k in exact fp32.
"""
import numpy as np

B, N, F, K, KNB, M, C = 32, 256, 128, 32, 16, 4, 8
RC2 = 36.0
NCONV = 3
DECAY = 0.5
UPD = 0.5
H, HP, HA, HE = 256, 128, 64, 128
NSP = 10
N_CORES = 8
MB = B // N_CORES  # molecules per core

_CACHE = {}

INPUT_SPECS = [
    ("species", (MB, N)), ("coords", (MB, N, 3)),
    ("emb", (NSP, F)), ("mu", (K,)), ("w", (K,)),
    ("Wc1", (K * F, H)), ("bc1", (H,)), ("Wc2", (H, F)), ("bc2", (F,)),
    ("Wp1", (F, HP)), ("bp1", (HP,)), ("Wp2", (HP, 16)), ("bp2", (16,)),
    ("Wa1", (2 * F + 1, HA)), ("ba1", (HA,)), ("Wa2", (HA, M)), ("ba2", (M,)),
    ("We1", (2 * F + 1, HE)), ("be1", (HE,)), ("We2", (HE, M * C)), ("be2", (M * C,)),
]


def build(mb=MB, trace_sim=False):
    import concourse.bass as bass
    import concourse.tile as tile
    import concourse.masks as masks
    from concourse import bacc, mybir
    from contextlib import ExitStack

    f32 = mybir.dt.float32
    f16 = mybir.dt.float16
    u32 = mybir.dt.uint32
    i32 = mybir.dt.int32
    Alu = mybir.AluOpType
    Act = mybir.ActivationFunctionType
    Ax = mybir.AxisListType

    nc = bacc.Bacc("TRN2", target_bir_lowering=False, debug=False,
                   num_devices=N_CORES)

    ins = {}
    for name, shape in INPUT_SPECS:
        shp = list(shape)
        if name == "species":
            shp = [mb, N]
        elif name == "coords":
            shp = [mb, N, 3]
        ins[name] = nc.dram_tensor(name, shp, f32, kind="ExternalInput").ap()

    o_iso = nc.dram_tensor("c_iso", [mb, N, 16], f32, kind="ExternalOutput").ap()
    o_an = nc.dram_tensor("c_aniso", [mb, N, KNB * M, C], f32,
                          kind="ExternalOutput").ap()

    with tile.TileContext(nc, trace_sim=trace_sim) as tc, ExitStack() as ctx:
        # ---------------- pools ----------------
        wpool = ctx.enter_context(tc.tile_pool(name="weights", bufs=1))
        stg = ctx.enter_context(tc.tile_pool(name="staging", bufs=2))
        gpool = ctx.enter_context(tc.tile_pool(name="gbasis", bufs=4))
        dpool = ctx.enter_context(tc.tile_pool(name="dists", bufs=2))
        xpool = ctx.enter_context(tc.tile_pool(name="xtiles", bufs=2))
        mpool = ctx.enter_context(tc.tile_pool(name="mconv", bufs=3))
        epool = ctx.enter_context(tc.tile_pool(name="edge", bufs=2))
        bigp = ctx.enter_context(tc.tile_pool(name="bigbcast", bufs=1))
        scr = ctx.enter_context(tc.tile_pool(name="scratch", bufs=3))
        outp = ctx.enter_context(tc.tile_pool(name="outstage", bufs=2))
        pm = ctx.enter_context(tc.tile_pool(name="psum_m", bufs=2, space="PSUM"))
        ph = ctx.enter_context(tc.tile_pool(name="psum_h", bufs=1, space="PSUM"))
        pt = ctx.enter_context(tc.tile_pool(name="psum_t", bufs=1, space="PSUM"))
        pe_ = ctx.enter_context(tc.tile_pool(name="psum_e", bufs=1, space="PSUM"))
        drp = ctx.enter_context(tc.tile_pool(name="dramscr", bufs=2, space="DRAM"))

        # ---------------- one-time weight prep ----------------
        ident16 = wpool.tile([128, 128], f16)
        masks.make_identity(nc, ident16[:])
        ident32 = wpool.tile([128, 128], f32)
        masks.make_identity(nc, ident32[:])

        def load_cast(dst_ap, src_ap, shape):
            s = stg.tile(list(shape), f32, tag="ldcast")
            nc.sync.dma_start(s[:], src_ap)
            nc.any.tensor_copy(dst_ap, s[:])

        # Wc1 [4096,256] -> fp16 [128, 32*256], chunk k at free offset k*256
        wc1h = wpool.tile([128, K * H], f16)
        for k in range(K):
            load_cast(wc1h[:, k * H:(k + 1) * H], ins["Wc1"][k * 128:(k + 1) * 128, :],
                      (128, H))
        # Wc2 [256,128] -> fp16 [128, 2*128]
        wc2h = wpool.tile([128, 2 * F], f16)
        for t in range(2):
            load_cast(wc2h[:, t * F:(t + 1) * F], ins["Wc2"][t * 128:(t + 1) * 128, :],
                      (128, F))
        embh = wpool.tile([NSP, F], f16)
        load_cast(embh[:], ins["emb"][:], (NSP, F))
        a1h = wpool.tile([128, HA], f16)
        load_cast(a1h[:], ins["Wa1"][0:F, :], (F, HA))
        a2h = wpool.tile([128, HA], f16)
        load_cast(a2h[:], ins["Wa1"][F:2 * F, :], (F, HA))
        e1h = wpool.tile([128, HE], f16)
        load_cast(e1h[:], ins["We1"][0:F, :], (F, HE))
        e2h = wpool.tile([128, HE], f16)
        load_cast(e2h[:], ins["We1"][F:2 * F, :], (F, HE))
        # Wa2 replicated x8 along output cols (m-major): [64, 32]
        wa2rep = wpool.tile([HA, M * C], f16)
        stg_wa2 = stg.tile([HA, M], f32, tag="ldcast")
        nc.sync.dma_start(stg_wa2[:], ins["Wa2"][:])
        nc.any.tensor_copy(wa2rep[:].rearrange("p (m c) -> p m c", c=C),
                           stg_wa2[:].unsqueeze(2).broadcast_to((HA, M, C)))
        we2h = wpool.tile([HE, M * C], f16)
        load_cast(we2h[:], ins["We2"][:], (HE, M * C))
        wp1h = wpool.tile([F, HP], f16)
        load_cast(wp1h[:], ins["Wp1"][:], (F, HP))
        wp2h = wpool.tile([HP, 16], f16)
        load_cast(wp2h[:], ins["Wp2"][:], (HP, 16))

        # bias columns (f32)
        def bias_col(src_ap, n):
            t = wpool.tile([n, 1], f32)
            nc.sync.dma_start(t[:], src_ap.unsqueeze(1))
            return t

        bc1c = wpool.tile([128, 2], f32)
        nc.sync.dma_start(bc1c[:], ins["bc1"].rearrange("(t p) -> p t", p=128))
        bc2c = bias_col(ins["bc2"], F)
        bp1c = bias_col(ins["bp1"], HP)
        bp2c = bias_col(ins["bp2"], 16)
        ba1c = bias_col(ins["ba1"], HA)
        ba2c = bias_col(ins["ba2"], M)
        be1c = bias_col(ins["be1"], HE)
        be2c = bias_col(ins["be2"], M * C)
        a3c = bias_col(ins["Wa1"][2 * F], HA)   # last row of Wa1
        e3c = bias_col(ins["We1"][2 * F], HE)

        eps9 = wpool.tile([128, 1], f32)
        nc.vector.memset(eps9[:], 1e-9)

        # mu / -1/w^2 broadcast [128, K]
        mu_b = wpool.tile([128, K], f32)
        nc.sync.dma_start(mu_b[:], ins["mu"].unsqueeze(0).to_broadcast((128, K)))
        w_b = wpool.tile([128, K], f32)
        nc.sync.dma_start(w_b[:], ins["w"].unsqueeze(0).to_broadcast((128, K)))
        w2 = wpool.tile([128, K], f32)
        nc.vector.tensor_mul(w2[:], w_b[:], w_b[:])
        iw2 = wpool.tile([128, K], f32)
        nc.vector.reciprocal(iw2[:], w2[:])
        ivw_b = wpool.tile([128, K], f32)
        nc.vector.reciprocal(ivw_b[:], w_b[:])
        nmuw_b = wpool.tile([128, K], f32)
        nc.vector.tensor_mul(nmuw_b[:], mu_b[:], ivw_b[:])
        nc.vector.tensor_scalar_mul(nmuw_b[:], nmuw_b[:], -1.0)

        # constant self-gather one-hots: oh_self[jt][j, (i,nb)] = (j_glob == i)
        oh_self = []
        for jt in range(2):
            t = wpool.tile([128, N * KNB], f16, name=f"ohself{jt}", tag=f"ohself{jt}")
            nc.gpsimd.memset(t[:], 0.0)
            nc.gpsimd.affine_select(
                out=t[:].rearrange("p (i nb) -> p i nb", nb=KNB),
                in_=t[:].rearrange("p (i nb) -> p i nb", nb=KNB),
                compare_op=Alu.not_equal, fill=1.0,
                base=128 * jt, pattern=[[-1, N], [0, KNB]], channel_multiplier=1,
            )
            oh_self.append(t)

        # constant self-gather one-hots: oh_self[jt][j, (i,nb)] = (j_glob == i)
        oh_self = []
        for jt in range(2):
            t = wpool.tile([128, N * KNB], f16)
            nc.gpsimd.memset(t[:], 0.0)
            nc.gpsimd.affine_select(
                out=t[:].rearrange("p (i nb) -> p i nb", nb=KNB),
                in_=t[:].rearrange("p (i nb) -> p i nb", nb=KNB),
                compare_op=Alu.not_equal, fill=1.0,
                base=128 * jt, pattern=[[-1, N], [0, KNB]], channel_multiplier=1,
            )
            oh_self.append(t)

        # iota columns
        iotaj = []
        for jt in range(2):
            ti = scr.tile([128, 1], i32, tag="iotai")
            nc.gpsimd.iota(ti[:], pattern=[[0, 1]], base=128 * jt,
                           channel_multiplier=1)
            tf = wpool.tile([128, 1], f32)
            nc.vector.tensor_copy(tf[:], ti[:])
            iotaj.append(tf)
        iota10i = scr.tile([NSP, 1], i32, tag="iotai")
        nc.gpsimd.iota(iota10i[:], pattern=[[0, 1]], base=0, channel_multiplier=1)
        iota10 = wpool.tile([NSP, 1], f32)
        nc.vector.tensor_copy(iota10[:], iota10i[:])

        # ---------------- per-molecule pipeline ----------------
        for mol in range(mb):
            # --- distances ---
            cb = dpool.tile([128, 3 * N], f32, tag="cb")  # coord rows bcast
            for a in range(3):
                nc.sync.dma_start(
                    cb[:, a * N:(a + 1) * N],
                    ins["coords"][mol, :, a].unsqueeze(0).to_broadcast((128, N)))
            d2 = []
            dS = []
            for it in range(2):
                ccol = dpool.tile([128, 3], f32, tag="ccol")
                nc.sync.dma_start(ccol[:], ins["coords"][mol, it * 128:(it + 1) * 128, :])
                dd = dpool.tile([128, N], f32, tag="d2")
                t1 = scr.tile([128, N], f32, tag="dscr")
                nc.vector.tensor_scalar(t1[:], cb[:, 0:N], ccol[:, 0:1], None,
                                        op0=Alu.subtract)
                nc.vector.tensor_mul(dd[:], t1[:], t1[:])
                for a in (1, 2):
                    t2 = scr.tile([128, N], f32, tag="dscr")
                    nc.vector.tensor_scalar(t2[:], cb[:, a * N:(a + 1) * N],
                                            ccol[:, a:a + 1], None, op0=Alu.subtract)
                    t3 = scr.tile([128, N], f32, tag="dscr")
                    nc.vector.tensor_mul(t3[:], t2[:], t2[:])
                    nc.vector.tensor_add(dd[:], dd[:], t3[:])
                # diagonal -> 1e9
                nc.gpsimd.affine_select(out=dd[:], in_=dd[:],
                                        compare_op=Alu.not_equal, fill=1e9,
                                        base=128 * it, pattern=[[-1, N]],
                                        channel_multiplier=1)
                d2.append(dd)
                ds_ = dpool.tile([128, N], f32, tag="dS")
                nc.scalar.activation(ds_[:], dd[:], Act.Sqrt, bias=eps9[:])
                dS.append(ds_)

            # --- gaussian basis g (fp16): arg = +((d-mu)/w)^2 then exp(-arg) ---
            K_ACT = 0  # ks whose arg is computed on ScalarE via Square trick
            g_sb = []
            for jt in range(2):
                dsh = scr.tile([128, N], f16, tag="dsh")
                nc.vector.tensor_copy(dsh[:], dS[jt][:])
                gt = gpool.tile([128, K * N], f16, tag="g")
                for k in range(K):
                    if k < K - K_ACT:
                        t1 = scr.tile([128, N], f16, tag="gscr")
                        if k % 2 == 1:
                            nc.gpsimd.tensor_scalar(t1[:], dsh[:], mu_b[:, k:k + 1],
                                                    ivw_b[:, k:k + 1],
                                                    op0=Alu.subtract, op1=Alu.mult)
                            nc.gpsimd.tensor_mul(gt[:, k * N:(k + 1) * N],
                                                 t1[:], t1[:])
                        else:
                            nc.vector.tensor_scalar(t1[:], dsh[:], mu_b[:, k:k + 1],
                                                    None, op0=Alu.subtract)
                            nc.vector.scalar_tensor_tensor(
                                gt[:, k * N:(k + 1) * N], t1[:], iw2[:, k:k + 1],
                                t1[:], op0=Alu.mult, op1=Alu.mult)
                    else:
                        nc.scalar.activation(gt[:, k * N:(k + 1) * N], dS[jt][:],
                                             Act.Square, scale=ivw_b[:, k:k + 1],
                                             bias=nmuw_b[:, k:k + 1])
                HALF = K * N // 2
                nc.scalar.activation(gt[:, 0:HALF], gt[:, 0:HALF], Act.Exp,
                                     scale=-1.0)
                nc.scalar.activation(gt[:, HALF:], gt[:, HALF:], Act.Exp,
                                     scale=-1.0)
                g_sb.append(gt)

            # --- top-16 neighbors per i-tile; pack (idx|dnb|mask) f16 ---
            # DRAM layout: 3 contiguous rows of 4096 (field-major)
            dcomb = drp.tile([3, N * KNB], f16, tag="dcomb")
            for it in range(2):
                neg = d2[it]
                nc.vector.tensor_scalar_mul(neg[:], d2[it][:], -1.0)
                vals = scr.tile([128, KNB], f32, tag="vals")
                idxs = scr.tile([128, KNB], u32, tag="idxs")
                nc.vector.max(vals[:, 0:8], neg[:])
                nc.vector.max_index(idxs[:, 0:8], vals[:, 0:8], neg[:])
                nc.vector.match_replace(neg[:], vals[:, 0:8], neg[:], -1e30)
                nc.vector.max(vals[:, 8:16], neg[:])
                nc.vector.max_index(idxs[:, 8:16], vals[:, 8:16], neg[:])
                d2sel = scr.tile([128, KNB], f32, tag="d2sel")
                nc.vector.tensor_scalar_mul(d2sel[:], vals[:], -1.0)
                comb = scr.tile([128, 3 * KNB], f16, tag="comb")
                nc.vector.tensor_copy(comb[:, 0:KNB], idxs[:])
                nc.scalar.activation(comb[:, KNB:2 * KNB], d2sel[:], Act.Sqrt,
                                     bias=eps9[:])
                nc.vector.tensor_scalar(comb[:, 2 * KNB:3 * KNB], d2sel[:], RC2,
                                        -60000.0, op0=Alu.is_ge, op1=Alu.mult)
                nc.sync.dma_start(
                    dcomb[:, it * 2048:(it + 1) * 2048]
                    .rearrange("f (p nb) -> p f nb", nb=KNB),
                    comb[:].rearrange("p (f nb) -> p f nb", nb=KNB))
            R = N * KNB  # 4096 edge rows
            idx_pb = bigp.tile([128, R], f16, tag="idxpb")
            nc.sync.dma_start(idx_pb[:], dcomb[0].unsqueeze(0)
                              .to_broadcast((128, R)))
            dnbrow = epool.tile([1, R], f16, tag="dnbrow")
            nc.gpsimd.dma_start(dnbrow[:], dcomb[1].unsqueeze(0))
            nmrow = epool.tile([1, R], f16, tag="nmrow")
            nc.gpsimd.dma_start(nmrow[:], dcomb[2].unsqueeze(0))

            # --- embedding x0 (xT layout [f, i]) ---
            spb = scr.tile([NSP, N], f32, tag="spb")
            nc.sync.dma_start(spb[:], ins["species"][mol].unsqueeze(0)
                              .to_broadcast((NSP, N)))
            ohsp = scr.tile([NSP, N], f16, tag="ohsp")
            nc.vector.tensor_scalar(ohsp[:], spb[:], iota10[:], None, op0=Alu.is_equal)
            ps0 = ph.tile([128, N], f32, tag="psh0")
            nc.tensor.matmul(ps0[:], embh[:], ohsp[:], start=True, stop=True)
            xT = xpool.tile([128, N], f32, tag="xT")
            nc.any.tensor_copy(xT[:], ps0[:])
            xTh = xpool.tile([128, N], f16, tag="xTh")
            nc.any.tensor_copy(xTh[:], ps0[:])

            def make_xrm(xTh_t):
                xrm = []
                for it in range(2):
                    pst = pt.tile([128, 128], f16, tag="pts")
                    nc.tensor.transpose(pst[:], xTh_t[:, it * 128:(it + 1) * 128],
                                        ident16[:])
                    xr = xpool.tile([128, F], f16, tag="xrm")
                    nc.any.tensor_copy(xr[:], pst[:])
                    xrm.append(xr)
                return xrm

            x_rm = make_xrm(xTh)

            # --- convolutions ---
            for cv in range(NCONV):
                ci = UPD * (DECAY ** cv)
                # off-critical-path: 0.5*x + ci*bc2 (ready long before psx)
                xhb = scr.tile([128, N], f32, tag="xhb")
                nc.vector.tensor_scalar(xhb[:], xT[:], 0.5, bc2ci[cv][:],
                                        op0=Alu.mult, op1=Alu.add)
                psh = []
                for t in range(2):
                    psh_t = ph.tile([128, N], f32, tag=f"psh{t}", name=f"psh{t}")
                    psh.append(psh_t)
                for kp in range(K // 2):
                    k0 = 2 * kp
                    psm = pm.tile([128, 2 * N], f32, tag="psm")
                    for dk in range(2):
                        k = k0 + dk
                        nc.tensor.matmul(psm[:, dk * N:(dk + 1) * N], x_rm[0][:],
                                         g_sb[0][:, k * N:(k + 1) * N],
                                         start=True, stop=False)
                        nc.tensor.matmul(psm[:, dk * N:(dk + 1) * N], x_rm[1][:],
                                         g_sb[1][:, k * N:(k + 1) * N],
                                         start=False, stop=True)
                    mk_t = mpool.tile([128, 2 * N], f16, tag="mk")
                    if kp % 4 != 3:
                        nc.vector.tensor_copy(mk_t[:], psm[:])
                    else:
                        nc.scalar.copy(mk_t[:], psm[:])
                    for dk in range(2):
                        k = k0 + dk
                        for t in range(2):
                            nc.tensor.matmul(
                                psh[t][:],
                                wc1h[:, k * H + t * 128:k * H + (t + 1) * 128],
                                mk_t[:, dk * N:(dk + 1) * N],
                                start=(k == 0), stop=(k == K - 1))
                silu_h = []
                for t in range(2):
                    sh = mpool.tile([128, N], f16, tag="siluh")
                    nc.scalar.activation(sh[:], psh[t][:], Act.Silu,
                                         bias=bc1c[:, t:t + 1])
                    silu_h.append(sh)
                psx = pm.tile([128, N], f32, tag="psm")
                for t in range(2):
                    nc.tensor.matmul(psx[:], wc2h[:, t * F:(t + 1) * F], silu_h[t][:],
                                     start=(t == 0), stop=(t == 1))
                xT_new = xpool.tile([128, N], f32, tag="xT")
                nc.vector.scalar_tensor_tensor(xT_new[:], psx[:], ci, xhb[:],
                                               op0=Alu.mult, op1=Alu.add)
                xT = xT_new
                xTh = xpool.tile([128, N], f16, tag="xTh")
                nc.any.tensor_copy(xTh[:], xT[:])
                if cv < NCONV - 1:
                    x_rm = make_xrm(xTh)

            # --- node pool head: c_iso ---
            psp = ph.tile([128, N], f32, tag="psh0")
            nc.tensor.matmul(psp[:], wp1h[:], xTh[:], start=True, stop=True)
            shp = mpool.tile([128, N], f16, tag="siluhp")
            nc.scalar.activation(shp[:], psp[:], Act.Silu, bias=bp1c[:])
            psc = pt.tile([16, N], f32, tag="pts")
            nc.tensor.matmul(psc[:], wp2h[:], shp[:], start=True, stop=True)
            cisoT = outp.tile([16, N], f32, tag="cisoT")
            nc.vector.tensor_scalar(cisoT[:], psc[:], bp2c[:16, :], None, op0=Alu.add)
            for it in range(2):
                pst = pt.tile([128, 16], f32, tag="pts")
                nc.tensor.transpose(pst[:], cisoT[:, it * 128:(it + 1) * 128],
                                    ident32[0:16, 0:16])
                crm = outp.tile([128, 16], f32, tag="cisorm")
                nc.any.tensor_copy(crm[:], pst[:])
                nc.sync.dma_start(o_iso[mol, it * 128:(it + 1) * 128, :], crm[:])

            # --- edge stage sources: row-major [j, ha/he] gather stationaries ---
            # all four y-matmuls per jt share one PSUM bank (sequential groups)
            y1a, y2a, y1e, y2e = [], [], [], []
            for jt in range(2):
                psy = pt.tile([128, 2 * HA + 2 * HE], f32, tag="pts", name="psy")
                off = 0
                for (wgt, hw) in ((a1h, HA), (a2h, HA), (e1h, HE), (e2h, HE)):
                    nc.tensor.matmul(psy[:, off:off + hw],
                                     xTh[:, jt * 128:(jt + 1) * 128], wgt[:],
                                     start=True, stop=True)
                    off += hw
                ybig = epool.tile([128, 2 * HA + 2 * HE], f16, tag="ybig")
                nc.vector.tensor_copy(ybig[:], psy[:])
                y1a.append(ybig[:, 0:HA])
                y2a.append(ybig[:, HA:2 * HA])
                y1e.append(ybig[:, 2 * HA:2 * HA + HE])
                y2e.append(ybig[:, 2 * HA + HE:])

            # --- edge chunks ---
            CH = 512
            NIA = CH // KNB  # atoms per chunk (32)
            o_an_flat = o_an[mol].rearrange("i (nb m) c2 -> (i nb) (m c2)", nb=KNB)
            for c in range(R // CH):
                sl = slice(c * CH, (c + 1) * CH)
                ohx = []
                for jt in range(2):
                    t = epool.tile([128, CH], f16, tag=f"ohx{jt}")
                    nc.gpsimd.tensor_scalar(t[:], idx_pb[:, sl], iotaj[jt][:], None,
                                            op0=Alu.is_equal)
                    ohx.append(t)
                pha = pe_.tile([HA, CH], f32, tag="pha")
                phe = pe_.tile([HE, CH], f32, tag="phe")
                for jt in range(2):
                    nc.tensor.matmul(pha[:], y1a[jt], oh_self[jt][:, sl],
                                     start=(jt == 0), stop=False)
                    nc.tensor.matmul(phe[:], y1e[jt], oh_self[jt][:, sl],
                                     start=(jt == 0), stop=False)
                for jt in range(2):
                    nc.tensor.matmul(pha[:], y2a[jt], ohx[jt][:],
                                     start=False, stop=False)
                    nc.tensor.matmul(phe[:], y2e[jt], ohx[jt][:],
                                     start=False, stop=False)
                nc.tensor.matmul(pha[:], a3row[:], dnbrow[:, sl],
                                 start=False, stop=True)
                nc.tensor.matmul(phe[:], e3row[:], dnbrow[:, sl],
                                 start=False, stop=True)
                sil_a = epool.tile([HA, CH], f16, tag="sila")
                nc.scalar.activation(sil_a[:], pha[:], Act.Silu, bias=ba1c[:])
                sil_e = epool.tile([HE, CH], f16, tag="sile")
                nc.scalar.activation(sil_e[:], phe[:], Act.Silu, bias=be1c[:])
                plg = pe_.tile([M * C, CH], f32, tag="pse", name="plg")
                nc.tensor.matmul(plg[:], wa2rep[:], sil_a[:], start=True, stop=False)
                nc.tensor.matmul(plg[:], ones32[:], nmrow[:, sl],
                                 start=False, stop=True)
                elog = epool.tile([M * C, CH], f32, tag="elog")
                nc.scalar.activation(elog[:], plg[:], Act.Exp, bias=ba2rep[:])
                pse = pe_.tile([M * C, CH], f32, tag="pse", name="pse")
                nc.tensor.matmul(pse[:], we2h[:], sil_e[:], start=True, stop=True)
                ssum = epool.tile([M * C, NIA], f32, tag="ssum")
                nc.vector.reduce_sum(
                    ssum[:], elog[:].rearrange("p (i nb) -> p i nb", nb=KNB),
                    axis=Ax.X)
                rec = epool.tile([M * C, NIA], f32, tag="rec")
                nc.vector.reciprocal(rec[:], ssum[:])
                attT = epool.tile([M * C, CH], f32, tag="attT")
                nc.vector.tensor_mul(
                    attT[:].rearrange("p (i nb) -> p i nb", nb=KNB),
                    elog[:].rearrange("p (i nb) -> p i nb", nb=KNB),
                    rec[:].unsqueeze(2).broadcast_to((M * C, NIA, KNB)))
                canT = epool.tile([M * C, CH], f32, tag="canT")
                nc.vector.scalar_tensor_tensor(canT[:], pse[:], be2c[:],
                                               attT[:], op0=Alu.add, op1=Alu.mult)
                pst4 = pt.tile([128, 4 * M * C], f32, tag="pts", name="pst4")
                for q in range(CH // 128):
                    nc.tensor.transpose(pst4[:, q * M * C:(q + 1) * M * C],
                                        canT[:, q * 128:(q + 1) * 128],
                                        ident32[0:M * C, 0:M * C])
                crm4 = outp.tile([128, 4 * M * C], f32, tag="canrm")
                nc.vector.tensor_copy(crm4[:], pst4[:])
                r0 = c * CH
                nc.sync.dma_start(
                    o_an_flat[r0:r0 + CH, :]
                    .rearrange("(q p) mc -> p q mc", q=4),
                    crm4[:].rearrange("p (q mc) -> p q mc", q=4))

    nc.compile()
    return nc


def _shard_inputs(inputs):
    arr = {k: np.ascontiguousarray(np.asarray(v), dtype=np.float32)
           for k, v in inputs.items()}
    in_maps = []
    for c in range(N_CORES):
        m = {}
        for name, _ in INPUT_SPECS:
            a = arr[name]
            if name in ("species", "coords"):
                m[name] = np.ascontiguousarray(a[c * MB:(c + 1) * MB])
            else:
                m[name] = a
        in_maps.append(m)
    return in_maps


def kernel(**inputs):
    from concourse.bass_utils import run_bass_kernel_spmd
    nc = _CACHE.get("nc")
    if nc is None:
        nc = build()
        _CACHE["nc"] = nc
    in_maps = _shard_inputs(inputs)
    res = run_bass_kernel_spmd(nc, in_maps, core_ids=list(range(N_CORES)))
    c_iso = np.concatenate([res.results[c]["c_iso"] for c in range(N_CORES)], axis=0)
    c_an = np.concatenate([res.results[c]["c_aniso"] for c in range(N_CORES)], axis=0)
    return c_iso, c_an
